# revision 5
# baseline (speedup 1.0000x reference)
"""DBRX attention block (B=1, T=2048, D=6144, 48 q heads / 8 kv heads, RoPE,
clamp, causal) as a Bass/Tile kernel on 8 Trainium2 NeuronCores.

Sharding: tensor-parallel over heads. Core c owns q heads [6c, 6c+6) and kv
head c (GQA groups align exactly: q head i uses kv head i//6). Each core
computes its slice of the QKV projection from the full x, runs RoPE + clamp +
causal attention for its 6 heads, then its partial output projection
(w_out row-sharded); the 8 partial outputs are summed on the host.

On-chip layout is feature-major ([feature, T]) throughout, so every matmul
is a clean [128 x 128] x [128 x 512] fp32r PE op at full rate:
  - qkvT[f, t]  = sum_d wqkvT[d, f] * xT[d, t]       (PSUM-accumulated over d)
  - RoPE via a pair-swap permutation matmul + DVE combine with cos/sin tables
  - scoresT[k, q] = sum_h kT[h, k] * qT[h, q], exp on ACT (no max-subtraction:
    scores are bounded by clamp at +-8, empirically |s| < ~25, exp fits fp32)
  - attnT[h, q] = sum_k v[k, h] * expT[k, q]; softmax denominator from a
    ones-vector matmul, applied via reciprocal + DMA partition-broadcast
  - outT[o, t] = sum_f w_o[f, o] * attnT[f, t]
"""

import math
import sys
from contextlib import ExitStack

import numpy as np

for _p in ("/opt/trn_rl_repo", "/root/.axon_site/_ro/trn_rl_repo"):
    if _p not in sys.path:
        sys.path.append(_p)

import concourse.bass as bass
import concourse.tile as tile
from concourse import bacc, mybir
from concourse.bass_utils import run_bass_kernel_spmd

F32 = mybir.dt.float32
F32R = mybir.dt.float32r
MULT = mybir.AluOpType.mult
ADD = mybir.AluOpType.add
MAX = mybir.AluOpType.max
MIN = mybir.AluOpType.min
EXP = mybir.ActivationFunctionType.Exp

N_CORES = 8
D_MODEL = 6144
N_HEADS = 48
N_KV_HEADS = 8
HEAD_DIM = 128
NQH = N_HEADS // N_CORES          # q heads per core = 6
T = 2048
CLAMP = 8.0
ROPE_BASE = 500000.0
SCALE = 1.0 / math.sqrt(HEAD_DIM)


def build_program(d_model=D_MODEL, t=T, nqh=NQH, chunk=6):
    """Build the per-core Bass program. Returns the compiled Bacc handle."""
    kd = d_model // 128            # contraction tiles for qkv projection
    tq = t // 512                  # 512-wide T quads
    tb = t // 128                  # 128-wide T blocks
    nf = nqh + 2                   # feature tiles: q heads + k + v
    qkv_cols = nf * 128

    nc = bacc.Bacc("TRN2", target_bir_lowering=False, debug=False)

    xT = nc.dram_tensor("xT", [d_model, t], F32, kind="ExternalInput").ap()
    wqkvT = nc.dram_tensor("wqkvT", [d_model, qkv_cols], F32, kind="ExternalInput").ap()
    w_o = nc.dram_tensor("w_o", [nqh * 128, d_model], F32, kind="ExternalInput").ap()
    cosf = nc.dram_tensor("cosf", [128, t], F32, kind="ExternalInput").ap()
    sinf = nc.dram_tensor("sinf", [128, t], F32, kind="ExternalInput").ap()
    maskT = nc.dram_tensor("maskT", [4, 128, 512], F32, kind="ExternalInput").ap()
    pswap = nc.dram_tensor("pswap", [128, 128], F32, kind="ExternalInput").ap()
    ident = nc.dram_tensor("ident", [128, 128], F32, kind="ExternalInput").ap()
    ones = nc.dram_tensor("ones", [128, 1], F32, kind="ExternalInput").ap()
    outT = nc.dram_tensor("outT", [d_model, t], F32, kind="ExternalOutput").ap()

    with tile.TileContext(nc) as tc, nc.allow_low_precision(reason="fp32r matmul pipeline"):
        with ExitStack() as octx:
            consts = octx.enter_context(tc.tile_pool(name="consts", bufs=1))
            accp = octx.enter_context(tc.tile_pool(name="accp", bufs=nf))

            sb_cos = consts.tile([128, t], F32, tag="cos")
            nc.sync.dma_start(sb_cos[:], cosf[:])
            sb_sin = consts.tile([128, t], F32, tag="sin")
            nc.sync.dma_start(sb_sin[:], sinf[:])
            sb_mask = consts.tile([128, 4, 512], F32, tag="mask")
            nc.sync.dma_start(sb_mask[:], maskT.transpose([1, 0, 2]))
            sb_pswap = consts.tile([128, 128], F32R, tag="pswap")
            nc.gpsimd.dma_start(sb_pswap[:], pswap[:])
            sb_ident = consts.tile([128, 128], F32R, tag="ident")
            nc.gpsimd.dma_start(sb_ident[:], ident[:])
            sb_ones = consts.tile([128, 1], F32R, tag="ones")
            nc.gpsimd.dma_start(sb_ones[:], ones[:])

            acc = [accp.tile([128, t], F32R, tag="acc", name=f"acc{i}") for i in range(nf)]

            # ---------------- Phase 1: qkvT = wqkvT^T-tiles @ xT ----------------
            with ExitStack() as ctx:
                xp = ctx.enter_context(tc.tile_pool(name="xp", bufs=chunk + 2))
                wp = ctx.enter_context(tc.tile_pool(name="wp", bufs=chunk + 2))
                psp = ctx.enter_context(tc.tile_pool(name="psp", bufs=2, space="PSUM"))
                n_chunks = (kd + chunk - 1) // chunk
                for c in range(n_chunks):
                    ks = list(range(c * chunk, min((c + 1) * chunk, kd)))
                    xts, wts = [], []
                    for k in ks:
                        xt = xp.tile([128, t], F32R, tag="xt")
                        nc.gpsimd.dma_start(xt[:], xT[k * 128:(k + 1) * 128, :])
                        xts.append(xt)
                        wt = wp.tile([128, qkv_cols], F32R, tag="wt")
                        nc.gpsimd.dma_start(wt[:], wqkvT[k * 128:(k + 1) * 128, :])
                        wts.append(wt)
                    for f in range(nf):
                        for q in range(tq):
                            sl = slice(q * 512, (q + 1) * 512)
                            ps = psp.tile([128, 512], F32, tag="ps")
                            for i in range(len(ks)):
                                nc.tensor.matmul(
                                    ps[:], wts[i][:, f * 128:(f + 1) * 128],
                                    xts[i][:, sl],
                                    start=(i == 0), stop=(i == len(ks) - 1),
                                )
                            if c == 0:
                                nc.vector.tensor_copy(acc[f][:, sl], ps[:])
                            else:
                                nc.vector.scalar_tensor_tensor(
                                    out=acc[f][:, sl], in0=ps[:], scalar=1.0,
                                    in1=acc[f][:, sl], op0=MULT, op1=ADD,
                                )

            # late pools: open only after phase 1 frees its SBUF
            attp = octx.enter_context(tc.tile_pool(name="attp", bufs=nqh))
            vtp = octx.enter_context(tc.tile_pool(name="vtp", bufs=1))
            attnT = [attp.tile([128, t], F32R, tag="attnT", name=f"attnT{i}") for i in range(nqh)]
            v_t = vtp.tile([128, tb, 128], F32R, tag="v_t")

            # ---------------- Phase 1.5: RoPE + clamp + v transpose ----------------
            with ExitStack() as ctx:
                rps = ctx.enter_context(tc.tile_pool(name="rps", bufs=2, space="PSUM"))
                tmp = ctx.enter_context(tc.tile_pool(name="rtmp", bufs=3))
                for f in range(nqh + 1):          # q heads and k head get RoPE
                    for q in range(tq):
                        sl = slice(q * 512, (q + 1) * 512)
                        pshuf = rps.tile([128, 512], F32, tag="pshuf")
                        nc.tensor.matmul(pshuf[:], sb_pswap[:], acc[f][:, sl],
                                         start=True, stop=True)
                        t1 = tmp.tile([128, 512], F32, tag="t1")
                        nc.vector.tensor_tensor(t1[:], acc[f][:, sl].bitcast(F32),
                                                sb_cos[:, sl], op=MULT)
                        t2 = tmp.tile([128, 512], F32, tag="t2")
                        nc.vector.tensor_tensor(t2[:], pshuf[:], sb_sin[:, sl], op=MULT)
                        t3 = tmp.tile([128, 512], F32, tag="t3")
                        nc.vector.tensor_tensor(t3[:], t1[:], t2[:], op=ADD)
                        nc.vector.tensor_scalar(acc[f][:, sl], t3[:], -CLAMP, CLAMP,
                                                op0=MAX, op1=MIN)
                iv = nqh + 1                      # v: clamp only, then transpose
                for q in range(tq):
                    sl = slice(q * 512, (q + 1) * 512)
                    t4 = tmp.tile([128, 512], F32, tag="t1")
                    nc.vector.tensor_scalar(t4[:], acc[iv][:, sl].bitcast(F32),
                                            -CLAMP, CLAMP, op0=MAX, op1=MIN)
                    nc.vector.tensor_copy(acc[iv][:, sl], t4[:])
                for j in range(tb):
                    pt = rps.tile([128, 128], F32R, tag="pt")
                    nc.tensor.transpose(pt[:], acc[iv][:, j * 128:(j + 1) * 128],
                                        sb_ident[:])
                    nc.vector.tensor_copy(v_t[:, j, :], pt[:])

            # ---------------- Phase 2: causal attention ----------------
            with ExitStack() as ctx:
                exps = ctx.enter_context(tc.tile_pool(name="exps", bufs=3))
                pss = ctx.enter_context(tc.tile_pool(name="pss", bufs=2, space="PSUM"))
                psa = ctx.enter_context(tc.tile_pool(name="psa", bufs=2, space="PSUM"))
                psm = ctx.enter_context(tc.tile_pool(name="psm", bufs=2, space="PSUM"))
                rcp = ctx.enter_context(tc.tile_pool(name="rcp", bufs=2))
                rcb = ctx.enter_context(tc.tile_pool(name="rcb", bufs=2))
                ik = nqh                          # k head feature tile
                for J in range(tq):
                    qsl = slice(J * 512, (J + 1) * 512)
                    nkb = 4 * J + 4
                    for h in range(nqh):
                        pa = psa.tile([128, 512], F32, tag="pa")
                        psum_s = psm.tile([1, 512], F32, tag="psum_s")
                        for kb in range(nkb):
                            ps = pss.tile([128, 512], F32, tag="ps")
                            nc.tensor.matmul(ps[:],
                                             acc[ik][:, kb * 128:(kb + 1) * 128],
                                             acc[h][:, qsl], start=True, stop=True)
                            if kb >= 4 * J:
                                nc.vector.tensor_tensor(
                                    ps[:], ps[:], sb_mask[:, kb - 4 * J, :], op=ADD)
                            ex = exps.tile([128, 512], F32R, tag="ex")
                            nc.scalar.activation(ex[:], ps[:], EXP, scale=SCALE)
                            nc.tensor.matmul(pa[:], v_t[:, kb, :], ex[:],
                                             start=(kb == 0), stop=(kb == nkb - 1))
                            nc.tensor.matmul(psum_s[:], sb_ones[:], ex[:],
                                             start=(kb == 0), stop=(kb == nkb - 1))
                        rc = rcp.tile([1, 512], F32R, tag="rc")
                        nc.vector.reciprocal(rc[:], psum_s[:])
                        rb = rcb.tile([128, 512], F32R, tag="rb")
                        nc.gpsimd.dma_start(
                            out=rb[:],
                            in_=bass.AP(tensor=rc.tensor, offset=rc.offset,
                                        ap=[[1, 1], [0, 128]] + rc.ap[1:]))
                        nc.vector.tensor_tensor(attnT[h][:, qsl], pa[:], rb[:],
                                                op=MULT)

            # ---------------- Phase 3: partial out projection ----------------
            with ExitStack() as ctx:
                wop = ctx.enter_context(tc.tile_pool(name="wop", bufs=4))
                outp = ctx.enter_context(tc.tile_pool(name="outp", bufs=4))
                pso = ctx.enter_context(tc.tile_pool(name="pso", bufs=2, space="PSUM"))
                for o in range(d_model // 128):
                    wo = wop.tile([128, nqh, 128], F32R, tag="wo")
                    src = w_o[:, o * 128:(o + 1) * 128].rearrange(
                        "(s p) o -> p s o", p=128)
                    nc.gpsimd.dma_start(wo[:], src)
                    for J in range(tq):
                        sl = slice(J * 512, (J + 1) * 512)
                        po = pso.tile([128, 512], F32, tag="po")
                        for s in range(nqh):
                            nc.tensor.matmul(po[:], wo[:, s, :], attnT[s][:, sl],
                                             start=(s == 0), stop=(s == nqh - 1))
                        ob = outp.tile([128, 512], F32, tag="ob")
                        nc.vector.tensor_copy(ob[:], po[:])
                        nc.sync.dma_start(outT[o * 128:(o + 1) * 128, sl], ob[:])

    nc.compile()
    return nc


def make_core_inputs(x, causal_mask, w_qkv, w_out, d_model=D_MODEL, t=T, nqh=NQH,
                     n_cores=N_CORES):
    """Host-side sharding: per-core input dicts for the SPMD program."""
    x2 = np.ascontiguousarray(x.reshape(t, d_model).T)      # [D, T]
    kv_base = N_HEADS * HEAD_DIM if d_model == D_MODEL else nqh * n_cores * HEAD_DIM
    n_kv = n_cores

    # RoPE tables in [head_dim, T] layout (interleaved-pair convention).
    hd = HEAD_DIM
    inv = 1.0 / ROPE_BASE ** (np.arange(0, hd, 2, dtype=np.float64) / hd)
    pos = np.arange(t, dtype=np.float64)
    freqs = pos[None, :] * inv[:, None]                     # [hd/2, T]
    cos = np.cos(freqs)
    sin = np.sin(freqs)
    cosf = np.empty((hd, t), dtype=np.float32)
    sinf = np.empty((hd, t), dtype=np.float32)
    cosf[0::2] = cos
    cosf[1::2] = cos
    sinf[0::2] = -sin                                       # row 2i:   -sin
    sinf[1::2] = sin                                        # row 2i+1: +sin

    cm = causal_mask.reshape(causal_mask.shape[-2], causal_mask.shape[-1])
    maskT = np.stack([np.ascontiguousarray(cm[0:512, m * 128:(m + 1) * 128].T)
                      for m in range(4)]).astype(np.float32)

    pswap = np.zeros((128, 128), dtype=np.float32)
    for i in range(0, 128, 2):
        pswap[i, i + 1] = 1.0
        pswap[i + 1, i] = 1.0
    ident = np.eye(128, dtype=np.float32)

    in_maps = []
    for c in range(n_cores):
        qrows = np.arange(c * nqh * 128, (c + 1) * nqh * 128)
        krows = np.arange(kv_base + c * 128, kv_base + (c + 1) * 128)
        vrows = np.arange(kv_base + n_kv * 128 + c * 128,
                          kv_base + n_kv * 128 + (c + 1) * 128)
        rows = np.concatenate([qrows, krows, vrows])
        wqkvT_c = np.ascontiguousarray(w_qkv[rows, :].T)    # [D, nqh*128+256]
        w_o_c = np.ascontiguousarray(w_out[:, qrows].T)     # [nqh*128, D]
        in_maps.append({
            "xT": x2, "wqkvT": wqkvT_c, "w_o": w_o_c,
            "cosf": cosf, "sinf": sinf, "maskT": maskT,
            "pswap": pswap, "ident": ident,
            "ones": np.ones((128, 1), dtype=np.float32),
        })
    return in_maps


_PROGRAM_CACHE = {}


def _get_program():
    key = (D_MODEL, T, NQH)
    if key not in _PROGRAM_CACHE:
        _PROGRAM_CACHE[key] = build_program()
    return _PROGRAM_CACHE[key]


def kernel(x, causal_mask, w_qkv, w_out):
    x = np.asarray(x, dtype=np.float32)
    causal_mask = np.asarray(causal_mask, dtype=np.float32)
    w_qkv = np.asarray(w_qkv, dtype=np.float32)
    w_out = np.asarray(w_out, dtype=np.float32)

    nc = _get_program()
    in_maps = make_core_inputs(x, causal_mask, w_qkv, w_out)
    res = run_bass_kernel_spmd(nc, in_maps, list(range(N_CORES)))
    outT = np.zeros((D_MODEL, T), dtype=np.float32)
    for c in range(N_CORES):
        outT += res.results[c]["outT"]
    return np.ascontiguousarray(outT.T).reshape(1, T, D_MODEL).astype(np.float32)
